# revision 24
# baseline (speedup 1.0000x reference)
"""GCN layer (gather -> scatter-mean -> linear -> relu) on 8 TRN2 NeuronCores.

Math: out = relu(segment_mean(x[src], dst) @ W.T + b), with rows whose
in-degree is 0 forced to 0.  The linear op commutes with the mean, so we
aggregate raw x first and apply the 128x128 weight afterwards.

Sharding: dst nodes are partitioned contiguously across the 8 cores; each
core receives the full x (for gathers) plus host-precomputed index arrays
for its edge shard.  Per 128-dst block, edges are gathered edge-partitioned
into SBUF with dma_gather (int16 indices -> the node space is split at 32768
into lo/hi regions).  The gather calls are spread round-robin over the 4
SWDGE queues so all four Q7 descriptor-generation pairs run concurrently
(the single-queue gather rate of ~7.9 ns/row was the kernel's bottleneck).

The gathered rows are cast to bf16 (ACT) and scatter-summed per 128-dst
block by PE matmuls against binary one-hot matrices generated on-device by
the DVE (iota == dst-local compare, batched 16 chunks per op), accumulated
in PSUM[feat, dst].  A bf16 matmul applies W.T, a K=1 matmul accumulates
count[d]*b, and ACT applies relu with a per-partition 1/count scale, which
turns raw sums into the mean and count*b into the bias (zeroing empty rows).
"""

import os
import sys
from contextlib import ExitStack

import numpy as np

for _p in ("/opt/trn_rl_repo", os.path.expanduser("~/.axon_site/_ro/trn_rl_repo")):
    if os.path.isdir(_p):
        if _p not in sys.path:
            sys.path.insert(0, _p)
        break

N_CORES = 8
P = 128
SPLIT = 32768  # dma_gather indices are int16
MAX_GROUP_CHUNKS = 96  # chunks (128 edges each) per gather group
OH_BATCH = 16  # one-hot chunks generated per DVE op
PAD_DL = 300.0  # dst-local sentinel for padded slots (matches no iota value)


class _Struct:
    pass


def _prep_structure(x_shape, edge_index):
    """Host-side bucketing of edges.  Returns the (core-invariant) static
    program structure plus per-core input arrays."""
    N, D = x_shape
    assert D == P, "kernel specialized to 128 features"
    assert N % N_CORES == 0
    NPC = N // N_CORES
    NB = -(-NPC // P)

    src = np.asarray(edge_index[0], dtype=np.int64)
    dst = np.asarray(edge_index[1], dtype=np.int64)
    counts = np.bincount(dst, minlength=N)

    core = dst // NPC
    drel = dst - core * NPC
    blk = drel // P
    dl = drel % P
    region = (src >= SPLIT).astype(np.int64)

    key = (core * NB + blk) * 2 + region
    order = np.argsort(key, kind="stable")
    ksort = key[order]
    ssort = src[order]
    dlsort = dl[order]
    nbuckets = N_CORES * NB * 2
    bcounts = np.bincount(ksort, minlength=nbuckets)
    boff = np.zeros(nbuckets + 1, np.int64)
    np.cumsum(bcounts, out=boff[1:])
    bc = bcounts.reshape(N_CORES, NB, 2)

    # Rank-matched slots: each core orders its blocks by per-region chunk
    # need (descending); program slot j holds each core's rank-j block, so
    # the static per-slot max over cores is tight.
    need = -(-bc // P)  # [core, block, region] chunk need
    perm = np.argsort(-(need.sum(axis=2) * 1000 + need[:, :, 0]), axis=1, kind="stable")
    # C[slot, region] = max over cores of that core's rank-slot block need
    need_sorted = np.take_along_axis(need, perm[:, :, None], axis=1)
    C = need_sorted.max(axis=0).astype(np.int64)  # [NB, 2]
    empty = C.sum(axis=1) == 0
    C[empty, 0] = 1

    # pack blocks into gather groups
    groups = []
    cur, curch = [], 0
    for b in range(NB):
        cb = int(C[b, 0] + C[b, 1])
        if cur and curch + cb > MAX_GROUP_CHUNKS:
            groups.append(cur)
            cur, curch = [], 0
        cur.append(b)
        curch += cb
    if cur:
        groups.append(cur)

    st = _Struct()
    st.N, st.D, st.NPC, st.NB = N, D, NPC, NB
    st.C = C
    st.groups = groups
    st.perm = perm  # [core, slot] -> physical block
    st.blk_rows = [min(P, NPC - b * P) for b in range(NB)]  # per physical block

    # static column layout
    st.n_lo = [int(C[bs, 0].sum()) * P for bs in groups]  # idxs per lo call
    st.n_hi = [int(C[bs, 1].sum()) * P for bs in groups]
    st.lo_col_off = np.concatenate([[0], np.cumsum([n // 16 for n in st.n_lo])])
    st.hi_col_off = np.concatenate([[0], np.cumsum([n // 16 for n in st.n_hi])])
    st.LO_COLS = int(st.lo_col_off[-1])
    st.HI_COLS = int(st.hi_col_off[-1])

    # call-chunk base per (g, region, b): position within the gather call
    st.call_base = {}
    for g, bs in enumerate(groups):
        for r in (0, 1):
            cb = 0
            for b in bs:
                st.call_base[(g, r, b)] = cb
                cb += int(C[b, r])
    # one-hot column per (g, region, b): consumption order (g, b, r)
    st.chunk_col = {}
    col = 0
    for g, bs in enumerate(groups):
        for b in bs:
            for r in (0, 1):
                st.chunk_col[(g, r, b)] = col
                col += int(C[b, r])
    st.TOT_CHUNKS = col

    # per-core input arrays
    per_core = []
    for c in range(N_CORES):
        lo_wraps, hi_wraps = [], []
        # dst-local index per (slot, chunk); PAD_DL marks padded slots
        dlarr = np.full((P, st.TOT_CHUNKS), PAD_DL, np.float32)
        node = c * NPC + np.arange(NB * P)
        valid = np.arange(NB * P) < NPC
        cnt = np.where(valid, counts[np.minimum(node, N - 1)], 0)
        rs = np.where(cnt > 0, 1.0 / np.maximum(cnt, 1), 0.0).astype(np.float32)
        rs2 = rs.reshape(NB, P)
        cnt2 = cnt.astype(np.float32).reshape(NB, P)

        for g, bs in enumerate(groups):
            for r, wraps in ((0, lo_wraps), (1, hi_wraps)):
                lists = []
                for b in bs:
                    pb = int(perm[c, b])  # physical block for this slot
                    k = (c * NB + pb) * 2 + r
                    s0, s1 = boff[k], boff[k + 1]
                    idxs = ssort[s0:s1] - (SPLIT if r else 0)
                    dls = dlsort[s0:s1]
                    n = s1 - s0
                    nch = int(C[b, r])
                    pad = nch * P - n
                    assert pad >= 0
                    if pad:
                        idxs = np.concatenate([idxs, np.zeros(pad, np.int64)])
                    lists.append(idxs)
                    e_pos = np.arange(n)
                    ch = st.chunk_col[(g, r, b)] + e_pos // P
                    dlarr[e_pos % P, ch] = dls
                if lists:
                    L = np.concatenate(lists)
                else:
                    L = np.zeros(0, np.int64)
                # wrapped[p, s] = L[s*16 + p]
                wraps.append(L.reshape(-1, 16).T.astype(np.int16))

        idx_lo = np.tile(np.concatenate(lo_wraps, axis=1), (P // 16, 1))
        idx_hi = np.tile(np.concatenate(hi_wraps, axis=1), (P // 16, 1))

        per_core.append(
            dict(
                idx_lo=np.ascontiguousarray(idx_lo),
                idx_hi=np.ascontiguousarray(idx_hi),
                dl=np.ascontiguousarray(dlarr),
                # counts and reciprocal scales in block-slot order
                cntrow=np.ascontiguousarray(cnt2[perm[c]].reshape(1, NB * P)),
                rscols=np.ascontiguousarray(rs2[perm[c]].T),  # [P, NB]
            )
        )
    return st, per_core


def _build_program(st):
    import concourse.bacc as bacc
    import concourse.tile as tile
    from concourse import mybir

    f32 = mybir.dt.float32
    bf16 = mybir.dt.bfloat16
    i16 = mybir.dt.int16
    Act = mybir.ActivationFunctionType
    Alu = mybir.AluOpType

    nc = bacc.Bacc(
        "TRN2", target_bir_lowering=False, debug=False, num_swdge_queues=4
    )
    x_t = nc.dram_tensor("x", [st.N, st.D], bf16, kind="ExternalInput")
    ilo_t = nc.dram_tensor("idx_lo", [P, st.LO_COLS], i16, kind="ExternalInput")
    ihi_t = nc.dram_tensor("idx_hi", [P, st.HI_COLS], i16, kind="ExternalInput")
    dl_t = nc.dram_tensor("dl", [P, st.TOT_CHUNKS], f32, kind="ExternalInput")
    cnt_t = nc.dram_tensor("cntrow", [1, st.NB * P], bf16, kind="ExternalInput")
    rs_t = nc.dram_tensor("rscols", [P, st.NB], f32, kind="ExternalInput")
    iota_t = nc.dram_tensor("iotaC", [P, OH_BATCH * P], bf16, kind="ExternalInput")
    brow_t = nc.dram_tensor("brow", [1, st.D], bf16, kind="ExternalInput")
    wt_t = nc.dram_tensor("wt", [st.D, st.D], bf16, kind="ExternalInput")
    out_t = nc.dram_tensor("out", [st.NB * P, st.D], f32, kind="ExternalOutput")

    with ExitStack() as ctx:
        tc = ctx.enter_context(tile.TileContext(nc))
        cpool = ctx.enter_context(tc.tile_pool(name="consts", bufs=1))
        gpool = ctx.enter_context(tc.tile_pool(name="gath", bufs=4))
        ohpool = ctx.enter_context(tc.tile_pool(name="oh", bufs=2))
        spool = ctx.enter_context(tc.tile_pool(name="sums", bufs=4))
        opool = ctx.enter_context(tc.tile_pool(name="outs", bufs=4))
        p1pool = ctx.enter_context(tc.tile_pool(name="ps1", bufs=4, space="PSUM"))
        p2pool = ctx.enter_context(tc.tile_pool(name="ps2", bufs=2, space="PSUM"))

        ilo_s = cpool.tile([P, st.LO_COLS], i16)
        nc.sync.dma_start(out=ilo_s[:], in_=ilo_t.ap()[:, :])
        ihi_s = cpool.tile([P, st.HI_COLS], i16)
        nc.sync.dma_start(out=ihi_s[:], in_=ihi_t.ap()[:, :])
        dl_s = cpool.tile([P, st.TOT_CHUNKS], f32)
        nc.sync.dma_start(out=dl_s[:], in_=dl_t.ap()[:, :])
        iota_s = cpool.tile([P, OH_BATCH * P], bf16)
        nc.sync.dma_start(out=iota_s[:], in_=iota_t.ap()[:, :])
        wtb_s = cpool.tile([st.D, st.D], bf16)
        nc.sync.dma_start(out=wtb_s[:], in_=wt_t.ap()[:, :])
        cntb_s = cpool.tile([1, st.NB * P], bf16)
        nc.sync.dma_start(out=cntb_s[:], in_=cnt_t.ap()[:, :])
        rs_s = cpool.tile([P, st.NB], f32)
        nc.sync.dma_start(out=rs_s[:], in_=rs_t.ap()[:, :])
        browb_s = cpool.tile([1, st.D], bf16)
        nc.sync.dma_start(out=browb_s[:], in_=brow_t.ap()[:, :])

        x_lo = x_t.ap()[0:SPLIT, :]
        x_hi = x_t.ap()[SPLIT : st.N, :]

        # warmup: the first SWDGE gather holds the engine synchronously for
        # its full generation time; make it a tiny throwaway so the real
        # calls pipeline across all four queues from the start.
        warm = cpool.tile([P, 2 * st.D], bf16)
        nc.gpsimd.dma_gather(
            out_ap=warm[:].rearrange("p (c e) -> p c e", e=st.D),
            in_ap=x_lo,
            idxs_ap=ilo_s[:, 0:16],
            num_idxs=256,
            num_idxs_reg=256,
            elem_size=st.D,
            single_packet=False,
            queue_num=0,
        )

        # one-hot tiles generated on-device in batches of OH_BATCH chunks
        oh_tiles = {}

        def oh_slice(k):
            bidx = k // OH_BATCH
            if bidx not in oh_tiles:
                w = min(OH_BATCH, st.TOT_CHUNKS - bidx * OH_BATCH)
                t = ohpool.tile([P, OH_BATCH * P], bf16, tag="oh", name=f"oh{bidx}")
                nc.vector.tensor_tensor(
                    out=t[:, : w * P].rearrange("p (c j) -> p c j", j=P),
                    in0=iota_s[:, : w * P].rearrange("p (c j) -> p c j", j=P),
                    in1=dl_s[:, bidx * OH_BATCH : bidx * OH_BATCH + w]
                    .unsqueeze(2)
                    .broadcast_to([P, w, P]),
                    op=Alu.is_equal,
                )
                oh_tiles[bidx] = t
            j = k - bidx * OH_BATCH
            return oh_tiles[bidx][:, j * P : (j + 1) * P]


        qload = [0, 0, 0, 0]
        qlast = []

        def next_q(n):
            cands = [q for q in range(4) if q not in qlast[-2:]]
            q = min(cands, key=lambda i: qload[i])
            qload[q] += n
            qlast.append(q)
            return q

        for g, bs in enumerate(st.groups):
            n_lo, n_hi = st.n_lo[g], st.n_hi[g]
            blo = bhi = None
            lo_tiles = {}
            if n_lo:
                nch = n_lo // P
                # split at the block boundary nearest the middle so each
                # block's chunks live in exactly one sub-call
                bnds = [st.call_base[(g, 0, b)] for b in bs[1:]] or [nch]
                half = min(bnds, key=lambda c: abs(c - nch // 2))
                if half in (0, nch):
                    half = nch // 2
                if g == 0 and nch > 12:
                    parts = [(0, 2), (2, 4), (4, 6), (6, 8)]
                    c = 8
                    while c < nch:
                        parts.append((c, min(c + 12, nch)))
                        c = min(c + 12, nch)
                elif g == 1 and nch > 12:
                    parts = []
                    c = 0
                    while c < nch:
                        parts.append((c, min(c + 12, nch)))
                        c = min(c + 12, nch)
                elif half:
                    parts = [(0, half), (half, nch)]
                else:
                    parts = [(0, nch)]
                for c0, c1 in parts:
                    t = gpool.tile(
                        [P, (c1 - c0) * st.D], bf16, tag=f"glo{min(c0, 9)}"
                    )
                    col0 = int(st.lo_col_off[g]) + c0 * 8
                    nn = (c1 - c0) * P
                    nc.gpsimd.dma_gather(
                        out_ap=t[:].rearrange("p (c e) -> p c e", e=st.D),
                        in_ap=x_lo,
                        idxs_ap=ilo_s[:, col0 : col0 + nn // 16],
                        num_idxs=nn,
                        num_idxs_reg=nn,
                        elem_size=st.D,
                        single_packet=False,
                        queue_num=next_q(nn),
                    )
                    for cc in range(c0, c1):
                        lo_tiles[cc] = (t, cc - c0)
            hi_tiles = {}
            if n_hi:
                hch = n_hi // P
                hbnds = [st.call_base[(g, 1, b)] for b in bs[1:]] or [hch]
                hhalf = min(hbnds, key=lambda c: abs(c - hch // 2))
                hparts = [(0, hhalf), (hhalf, hch)] if 0 < hhalf < hch else [(0, hch)]
                for hi_i, (hc0, hc1) in enumerate(hparts):
                    bhi = gpool.tile(
                        [P, (hc1 - hc0) * st.D], bf16, tag=f"ghi{hi_i}"
                    )
                    col0 = int(st.hi_col_off[g]) + hc0 * 8
                    nn = (hc1 - hc0) * P
                    nc.gpsimd.dma_gather(
                        out_ap=bhi[:].rearrange("p (c e) -> p c e", e=st.D),
                        in_ap=x_hi,
                        idxs_ap=ihi_s[:, col0 : col0 + nn // 16],
                        num_idxs=nn,
                        num_idxs_reg=nn,
                        elem_size=st.D,
                        single_packet=False,
                        queue_num=next_q(nn),
                    )
                    for cc in range(hc0, hc1):
                        hi_tiles[cc] = (bhi, cc - hc0)

            for b in bs:
                total = int(st.C[b, 0] + st.C[b, 1])
                ps1 = p1pool.tile([P, P], f32, tag="ps1")
                k = 0
                for r, tmap in ((0, lo_tiles), (1, hi_tiles)):
                    for j in range(int(st.C[b, r])):
                        cc = st.call_base[(g, r, b)] + j
                        col = st.chunk_col[(g, r, b)] + j
                        t, tc = tmap[cc]
                        nc.tensor.matmul(
                            ps1[:],
                            lhsT=t[:, tc * st.D : (tc + 1) * st.D],
                            rhs=oh_slice(col),
                            start=(k == 0),
                            stop=(k == total - 1),
                        )
                        k += 1

                sums = spool.tile([P, P], bf16, tag="sums")
                nc.vector.tensor_copy(out=sums[:], in_=ps1[:])
                ps2 = p2pool.tile([P, st.D], f32, tag="ps2")
                nc.tensor.matmul(
                    ps2[:], lhsT=sums[:], rhs=wtb_s[:], start=True, stop=False
                )
                nc.tensor.matmul(
                    ps2[:],
                    lhsT=cntb_s[:1, b * P : (b + 1) * P],
                    rhs=browb_s[:1, :],
                    start=False,
                    stop=True,
                )
                of = opool.tile([P, st.D], f32, tag="of")
                nc.scalar.activation(
                    of[:], ps2[:], Act.Relu, scale=rs_s[:, b : b + 1]
                )
                nc.sync.dma_start(
                    out=out_t.ap()[b * P : (b + 1) * P, :], in_=of[:, :]
                )

    nc.compile()
    return nc


def emulate(x, edge_index, W, b):
    """Pure-numpy emulation of the device program (for validation)."""
    import ml_dtypes

    bf = ml_dtypes.bfloat16
    x = np.asarray(x, np.float32)
    st, per_core = _prep_structure(x.shape, edge_index)
    wt = np.ascontiguousarray(np.asarray(W, np.float32).T).astype(bf).astype(np.float32)
    brow = np.asarray(b, np.float32).astype(bf).astype(np.float32)
    outs = []
    xr = [x[:SPLIT], x[SPLIT:]]
    iota = np.arange(P, dtype=np.float32)
    for c in range(N_CORES):
        a = per_core[c]
        out_c = np.zeros((st.NPC, st.D), np.float32)
        for g, bs in enumerate(st.groups):
            gath = []
            for r, (arr, offs) in enumerate(
                ((a["idx_lo"], st.lo_col_off), (a["idx_hi"], st.hi_col_off))
            ):
                n = (st.n_lo, st.n_hi)[r][g]
                wrapped = arr[:16, int(offs[g]) : int(offs[g]) + n // 16]
                unwrapped = wrapped.T.reshape(-1).astype(np.int64)
                gath.append(
                    xr[r][unwrapped].astype(bf).astype(np.float32)
                    if n
                    else np.zeros((0, st.D), np.float32)
                )
            for bi in bs:
                S = np.zeros((st.D, P), np.float32)
                for r in (0, 1):
                    for j in range(int(st.C[bi, r])):
                        cc = st.call_base[(g, r, bi)] + j
                        col = st.chunk_col[(g, r, bi)] + j
                        got = gath[r][cc * P : (cc + 1) * P]  # [128e, D]
                        oh = (iota[None, :] == a["dl"][:, col][:, None]).astype(
                            np.float32
                        )
                        S += got.T @ oh
                Sb = S.astype(bf).astype(np.float32)
                cnt = a["cntrow"][0, bi * P : (bi + 1) * P].astype(bf).astype(np.float32)
                z = Sb.T @ wt + cnt[:, None] * brow[None, :]
                rs = a["rscols"][:, bi]
                oo = np.maximum(z * rs[:, None], 0.0)
                pb = int(st.perm[c, bi])
                nr = st.blk_rows[pb]
                out_c[pb * P : pb * P + nr] = oo[:nr]
        outs.append(out_c)
    return np.concatenate(outs, axis=0)[: x.shape[0]]


_RUN_INFO = {}


def _install_ntff_hook():
    """Recreate the antenv.axon_hooks NTFF profile hook via ctypes on the
    injected axon PJRT .so (the agent image's antenv lacks axon_hooks)."""
    import contextlib
    import ctypes
    import types

    try:
        from antenv.axon_hooks import get_axon_ntff_profile_hook  # noqa: F401

        return True
    except ImportError:
        pass

    so_path = "/opt/axon/libaxon_pjrt.so"
    if not os.path.exists(so_path):
        return False
    lib = ctypes.CDLL(so_path)
    if not hasattr(lib, "axon_start_nrt_profile"):
        return False
    lib.axon_start_nrt_profile.argtypes = [
        ctypes.POINTER(ctypes.c_int64),
        ctypes.c_size_t,
    ]
    lib.axon_start_nrt_profile.restype = ctypes.c_int64
    lib.axon_stop_nrt_profile.argtypes = [ctypes.c_char_p]
    lib.axon_stop_nrt_profile.restype = ctypes.c_int64

    @contextlib.contextmanager
    def _hook(output_dir, device_ids):
        import jax

        jax.devices()
        if device_ids:
            ids = (ctypes.c_int64 * len(device_ids))(*device_ids)
            rc = lib.axon_start_nrt_profile(ids, len(device_ids))
        else:
            rc = lib.axon_start_nrt_profile(None, 0)
        if rc != 0:
            raise RuntimeError(f"axon_start_nrt_profile rc={rc}")
        try:
            yield
        finally:
            n = lib.axon_stop_nrt_profile(str(output_dir).encode())
            print(f"ntff profile: {n} file(s) written to {output_dir}")

    mod = types.ModuleType("antenv.axon_hooks")
    mod.get_axon_ntff_profile_hook = lambda: _hook
    mod.set_axon_ntff_profile_hook = lambda h: None
    import antenv

    sys.modules["antenv.axon_hooks"] = mod
    antenv.axon_hooks = mod

    # avoid remote artifact uploads during profile post-processing
    from concourse import bass_utils

    bass_utils.upload_artifacts = lambda tmpdir: tmpdir
    return True


def kernel(x, edge_index, W, b, _trace=False):
    from concourse.bass_utils import run_bass_kernel_spmd

    import ml_dtypes as _mld

    x = np.ascontiguousarray(np.asarray(x, dtype=np.float32).astype(_mld.bfloat16))
    edge_index = np.asarray(edge_index)
    st, per_core = _prep_structure(x.shape, edge_index)
    wt = np.ascontiguousarray(np.asarray(W, np.float32).T.astype(_mld.bfloat16))
    brow = np.ascontiguousarray(
        np.asarray(b, np.float32).reshape(1, -1).astype(_mld.bfloat16)
    )
    import ml_dtypes

    iotaC = np.ascontiguousarray(
        np.tile(np.arange(P, dtype=np.float32), (P, OH_BATCH)).astype(
            ml_dtypes.bfloat16
        )
    )

    nc = _build_program(st)
    in_maps = []
    for c in range(N_CORES):
        a = per_core[c]
        in_maps.append(
            dict(
                x=x,
                idx_lo=a["idx_lo"],
                idx_hi=a["idx_hi"],
                dl=a["dl"],
                cntrow=a["cntrow"].astype(_mld.bfloat16),
                rscols=a["rscols"],
                iotaC=iotaC,
                brow=brow,
                wt=wt,
            )
        )
    if _trace:
        _trace = _install_ntff_hook()
    import tempfile

    tmpdir = tempfile.mkdtemp(prefix="gcn_bass_")
    try:
        res = run_bass_kernel_spmd(
            nc, in_maps, core_ids=list(range(N_CORES)), trace=_trace, tmpdir=tmpdir
        )
    except Exception:
        if not _trace:
            raise
        sys.stderr.write("trace run failed; retrying without trace\n")
        res = run_bass_kernel_spmd(nc, in_maps, core_ids=list(range(N_CORES)))
    _RUN_INFO["exec_time_ns"] = res.exec_time_ns
    _RUN_INFO["profile_json"] = res.profile_json
    _RUN_INFO["tmpdir"] = tmpdir
    out = np.zeros((st.N, st.D), np.float32)
    for c in range(N_CORES):
        oc = res.results[c]["out"]
        for j in range(st.NB):
            pb = int(st.perm[c, j])
            nr = st.blk_rows[pb]
            out[c * st.NPC + pb * P : c * st.NPC + pb * P + nr] = oc[
                j * P : j * P + nr
            ]
    return out


# revision 25
# speedup vs baseline: 1.0323x; 1.0323x over previous
"""GCN layer (gather -> scatter-mean -> linear -> relu) on 8 TRN2 NeuronCores.

Math: out = relu(segment_mean(x[src], dst) @ W.T + b), with rows whose
in-degree is 0 forced to 0.  The linear op commutes with the mean, so we
aggregate raw x first and apply the 128x128 weight afterwards.

Sharding: dst nodes are partitioned contiguously across the 8 cores; each
core receives the full x (for gathers) plus host-precomputed index arrays
for its edge shard.  Per 128-dst block, edges are gathered edge-partitioned
into SBUF with dma_gather (int16 indices -> the node space is split at 32768
into lo/hi regions).  The gather calls are spread round-robin over the 4
SWDGE queues so all four Q7 descriptor-generation pairs run concurrently
(the single-queue gather rate of ~7.9 ns/row was the kernel's bottleneck).

The gathered rows are cast to bf16 (ACT) and scatter-summed per 128-dst
block by PE matmuls against binary one-hot matrices generated on-device by
the DVE (iota == dst-local compare, batched 16 chunks per op), accumulated
in PSUM[feat, dst].  A bf16 matmul applies W.T, a K=1 matmul accumulates
count[d]*b, and ACT applies relu with a per-partition 1/count scale, which
turns raw sums into the mean and count*b into the bias (zeroing empty rows).
"""

import os
import sys
from contextlib import ExitStack

import numpy as np

for _p in ("/opt/trn_rl_repo", os.path.expanduser("~/.axon_site/_ro/trn_rl_repo")):
    if os.path.isdir(_p):
        if _p not in sys.path:
            sys.path.insert(0, _p)
        break

N_CORES = 8
P = 128
SPLIT = 32768  # dma_gather indices are int16
MAX_GROUP_CHUNKS = 96  # chunks (128 edges each) per gather group
OH_BATCH = 16  # one-hot chunks generated per DVE op
PAD_DL = 300.0  # dst-local sentinel for padded slots (matches no iota value)


class _Struct:
    pass


def _prep_structure(x_shape, edge_index):
    """Host-side bucketing of edges.  Returns the (core-invariant) static
    program structure plus per-core input arrays."""
    N, D = x_shape
    assert D == P, "kernel specialized to 128 features"
    assert N % N_CORES == 0
    NPC = N // N_CORES
    NB = -(-NPC // P)

    src = np.asarray(edge_index[0], dtype=np.int64)
    dst = np.asarray(edge_index[1], dtype=np.int64)
    counts = np.bincount(dst, minlength=N)

    core = dst // NPC
    drel = dst - core * NPC
    blk = drel // P
    dl = drel % P
    region = (src >= SPLIT).astype(np.int64)

    key = (core * NB + blk) * 2 + region
    order = np.argsort(key, kind="stable")
    ksort = key[order]
    ssort = src[order]
    dlsort = dl[order]
    nbuckets = N_CORES * NB * 2
    bcounts = np.bincount(ksort, minlength=nbuckets)
    boff = np.zeros(nbuckets + 1, np.int64)
    np.cumsum(bcounts, out=boff[1:])
    bc = bcounts.reshape(N_CORES, NB, 2)

    # Rank-matched slots: each core orders its blocks by per-region chunk
    # need (descending); program slot j holds each core's rank-j block, so
    # the static per-slot max over cores is tight.
    need = -(-bc // P)  # [core, block, region] chunk need
    perm = np.argsort(-(need.sum(axis=2) * 1000 + need[:, :, 0]), axis=1, kind="stable")
    # C[slot, region] = max over cores of that core's rank-slot block need
    need_sorted = np.take_along_axis(need, perm[:, :, None], axis=1)
    C = need_sorted.max(axis=0).astype(np.int64)  # [NB, 2]
    empty = C.sum(axis=1) == 0
    C[empty, 0] = 1

    # pack blocks into gather groups
    groups = []
    cur, curch = [], 0
    for b in range(NB):
        cb = int(C[b, 0] + C[b, 1])
        if cur and curch + cb > MAX_GROUP_CHUNKS:
            groups.append(cur)
            cur, curch = [], 0
        cur.append(b)
        curch += cb
    if cur:
        groups.append(cur)

    st = _Struct()
    st.N, st.D, st.NPC, st.NB = N, D, NPC, NB
    st.C = C
    st.groups = groups
    st.perm = perm  # [core, slot] -> physical block
    st.blk_rows = [min(P, NPC - b * P) for b in range(NB)]  # per physical block

    # static column layout
    st.n_lo = [int(C[bs, 0].sum()) * P for bs in groups]  # idxs per lo call
    st.n_hi = [int(C[bs, 1].sum()) * P for bs in groups]
    st.lo_col_off = np.concatenate([[0], np.cumsum([n // 16 for n in st.n_lo])])
    st.hi_col_off = np.concatenate([[0], np.cumsum([n // 16 for n in st.n_hi])])
    st.LO_COLS = int(st.lo_col_off[-1])
    st.HI_COLS = int(st.hi_col_off[-1])

    # call-chunk base per (g, region, b): position within the gather call
    st.call_base = {}
    for g, bs in enumerate(groups):
        for r in (0, 1):
            cb = 0
            for b in bs:
                st.call_base[(g, r, b)] = cb
                cb += int(C[b, r])
    # one-hot column per (g, region, b): consumption order (g, b, r)
    st.chunk_col = {}
    col = 0
    for g, bs in enumerate(groups):
        for b in bs:
            for r in (0, 1):
                st.chunk_col[(g, r, b)] = col
                col += int(C[b, r])
    st.TOT_CHUNKS = col

    # per-core input arrays
    per_core = []
    for c in range(N_CORES):
        lo_wraps, hi_wraps = [], []
        # dst-local index per (slot, chunk); PAD_DL marks padded slots
        dlarr = np.full((P, st.TOT_CHUNKS), PAD_DL, np.float32)
        node = c * NPC + np.arange(NB * P)
        valid = np.arange(NB * P) < NPC
        cnt = np.where(valid, counts[np.minimum(node, N - 1)], 0)
        rs = np.where(cnt > 0, 1.0 / np.maximum(cnt, 1), 0.0).astype(np.float32)
        rs2 = rs.reshape(NB, P)
        cnt2 = cnt.astype(np.float32).reshape(NB, P)

        for g, bs in enumerate(groups):
            for r, wraps in ((0, lo_wraps), (1, hi_wraps)):
                lists = []
                for b in bs:
                    pb = int(perm[c, b])  # physical block for this slot
                    k = (c * NB + pb) * 2 + r
                    s0, s1 = boff[k], boff[k + 1]
                    idxs = ssort[s0:s1] - (SPLIT if r else 0)
                    dls = dlsort[s0:s1]
                    n = s1 - s0
                    nch = int(C[b, r])
                    pad = nch * P - n
                    assert pad >= 0
                    if pad:
                        idxs = np.concatenate([idxs, np.zeros(pad, np.int64)])
                    lists.append(idxs)
                    e_pos = np.arange(n)
                    ch = st.chunk_col[(g, r, b)] + e_pos // P
                    dlarr[e_pos % P, ch] = dls
                if lists:
                    L = np.concatenate(lists)
                else:
                    L = np.zeros(0, np.int64)
                # wrapped[p, s] = L[s*16 + p]
                wraps.append(L.reshape(-1, 16).T.astype(np.int16))

        idx_lo = np.tile(np.concatenate(lo_wraps, axis=1), (P // 16, 1))
        idx_hi = np.tile(np.concatenate(hi_wraps, axis=1), (P // 16, 1))

        per_core.append(
            dict(
                idx_lo=np.ascontiguousarray(idx_lo),
                idx_hi=np.ascontiguousarray(idx_hi),
                dl=np.ascontiguousarray(dlarr),
                # counts and reciprocal scales in block-slot order
                cntrow=np.ascontiguousarray(cnt2[perm[c]].reshape(1, NB * P)),
                rscols=np.ascontiguousarray(rs2[perm[c]].T),  # [P, NB]
            )
        )
    return st, per_core


def _build_program(st):
    import concourse.bacc as bacc
    import concourse.tile as tile
    from concourse import mybir

    f32 = mybir.dt.float32
    bf16 = mybir.dt.bfloat16
    i16 = mybir.dt.int16
    Act = mybir.ActivationFunctionType
    Alu = mybir.AluOpType

    nc = bacc.Bacc(
        "TRN2", target_bir_lowering=False, debug=False, num_swdge_queues=4
    )
    x_t = nc.dram_tensor("x", [st.N, st.D], bf16, kind="ExternalInput")
    ilo_t = nc.dram_tensor("idx_lo", [P, st.LO_COLS], i16, kind="ExternalInput")
    ihi_t = nc.dram_tensor("idx_hi", [P, st.HI_COLS], i16, kind="ExternalInput")
    dl_t = nc.dram_tensor("dl", [P, st.TOT_CHUNKS], f32, kind="ExternalInput")
    cnt_t = nc.dram_tensor("cntrow", [1, st.NB * P], bf16, kind="ExternalInput")
    rs_t = nc.dram_tensor("rscols", [P, st.NB], f32, kind="ExternalInput")
    iota_t = nc.dram_tensor("iotaC", [P, OH_BATCH * P], bf16, kind="ExternalInput")
    brow_t = nc.dram_tensor("brow", [1, st.D], bf16, kind="ExternalInput")
    wt_t = nc.dram_tensor("wt", [st.D, st.D], bf16, kind="ExternalInput")
    out_t = nc.dram_tensor("out", [st.NB * P, st.D], f32, kind="ExternalOutput")

    with ExitStack() as ctx:
        tc = ctx.enter_context(tile.TileContext(nc))
        cpool = ctx.enter_context(tc.tile_pool(name="consts", bufs=1))
        gpool = ctx.enter_context(tc.tile_pool(name="gath", bufs=4))
        ohpool = ctx.enter_context(tc.tile_pool(name="oh", bufs=2))
        spool = ctx.enter_context(tc.tile_pool(name="sums", bufs=4))
        opool = ctx.enter_context(tc.tile_pool(name="outs", bufs=4))
        p1pool = ctx.enter_context(tc.tile_pool(name="ps1", bufs=4, space="PSUM"))
        p2pool = ctx.enter_context(tc.tile_pool(name="ps2", bufs=2, space="PSUM"))

        ilo_s = cpool.tile([P, st.LO_COLS], i16)
        nc.sync.dma_start(out=ilo_s[:], in_=ilo_t.ap()[:, :])
        ihi_s = cpool.tile([P, st.HI_COLS], i16)
        nc.sync.dma_start(out=ihi_s[:], in_=ihi_t.ap()[:, :])
        dl_s = cpool.tile([P, st.TOT_CHUNKS], f32)
        nc.sync.dma_start(out=dl_s[:], in_=dl_t.ap()[:, :])
        iota_s = cpool.tile([P, OH_BATCH * P], bf16)
        nc.sync.dma_start(out=iota_s[:], in_=iota_t.ap()[:, :])
        wtb_s = cpool.tile([st.D, st.D], bf16)
        nc.sync.dma_start(out=wtb_s[:], in_=wt_t.ap()[:, :])
        cntb_s = cpool.tile([1, st.NB * P], bf16)
        nc.sync.dma_start(out=cntb_s[:], in_=cnt_t.ap()[:, :])
        rs_s = cpool.tile([P, st.NB], f32)
        nc.sync.dma_start(out=rs_s[:], in_=rs_t.ap()[:, :])
        browb_s = cpool.tile([1, st.D], bf16)
        nc.sync.dma_start(out=browb_s[:], in_=brow_t.ap()[:, :])

        x_lo = x_t.ap()[0:SPLIT, :]
        x_hi = x_t.ap()[SPLIT : st.N, :]

        # warmup: the first SWDGE gather holds the engine synchronously for
        # its full generation time; make it a tiny throwaway so the real
        # calls pipeline across all four queues from the start.
        warm = cpool.tile([P, 2 * st.D], bf16)
        nc.gpsimd.dma_gather(
            out_ap=warm[:].rearrange("p (c e) -> p c e", e=st.D),
            in_ap=x_lo,
            idxs_ap=ilo_s[:, 0:16],
            num_idxs=256,
            num_idxs_reg=256,
            elem_size=st.D,
            single_packet=False,
            queue_num=0,
        )

        # one-hot tiles generated on-device in batches of OH_BATCH chunks
        oh_tiles = {}

        def oh_slice(k):
            bidx = k // OH_BATCH
            if bidx not in oh_tiles:
                w = min(OH_BATCH, st.TOT_CHUNKS - bidx * OH_BATCH)
                t = ohpool.tile([P, OH_BATCH * P], bf16, tag="oh", name=f"oh{bidx}")
                nc.vector.tensor_tensor(
                    out=t[:, : w * P].rearrange("p (c j) -> p c j", j=P),
                    in0=iota_s[:, : w * P].rearrange("p (c j) -> p c j", j=P),
                    in1=dl_s[:, bidx * OH_BATCH : bidx * OH_BATCH + w]
                    .unsqueeze(2)
                    .broadcast_to([P, w, P]),
                    op=Alu.is_equal,
                )
                oh_tiles[bidx] = t
            j = k - bidx * OH_BATCH
            return oh_tiles[bidx][:, j * P : (j + 1) * P]


        qload = [0, 0, 0, 0]
        qlast = []

        def next_q(n):
            cands = [q for q in range(4) if q not in qlast[-2:]]
            q = min(cands, key=lambda i: qload[i])
            qload[q] += n
            qlast.append(q)
            return q

        for g, bs in enumerate(st.groups):
            n_lo, n_hi = st.n_lo[g], st.n_hi[g]
            blo = bhi = None
            lo_tiles = {}
            if n_lo:
                nch = n_lo // P
                # split at the block boundary nearest the middle so each
                # block's chunks live in exactly one sub-call
                bnds = [st.call_base[(g, 0, b)] for b in bs[1:]] or [nch]
                half = min(bnds, key=lambda c: abs(c - nch // 2))
                if half in (0, nch):
                    half = nch // 2
                if g == 0 and nch > 12:
                    parts = [(0, 2), (2, 4), (4, 6), (6, 8), (8, half), (half, nch)]
                elif half:
                    parts = [(0, half), (half, nch)]
                else:
                    parts = [(0, nch)]
                for c0, c1 in parts:
                    t = gpool.tile(
                        [P, (c1 - c0) * st.D], bf16, tag=f"glo{min(c0, 9)}"
                    )
                    col0 = int(st.lo_col_off[g]) + c0 * 8
                    nn = (c1 - c0) * P
                    nc.gpsimd.dma_gather(
                        out_ap=t[:].rearrange("p (c e) -> p c e", e=st.D),
                        in_ap=x_lo,
                        idxs_ap=ilo_s[:, col0 : col0 + nn // 16],
                        num_idxs=nn,
                        num_idxs_reg=nn,
                        elem_size=st.D,
                        single_packet=False,
                        queue_num=next_q(nn),
                    )
                    for cc in range(c0, c1):
                        lo_tiles[cc] = (t, cc - c0)
            hi_tiles = {}
            if n_hi:
                hch = n_hi // P
                hbnds = [st.call_base[(g, 1, b)] for b in bs[1:]] or [hch]
                hhalf = min(hbnds, key=lambda c: abs(c - hch // 2))
                hparts = [(0, hhalf), (hhalf, hch)] if 0 < hhalf < hch else [(0, hch)]
                for hi_i, (hc0, hc1) in enumerate(hparts):
                    bhi = gpool.tile(
                        [P, (hc1 - hc0) * st.D], bf16, tag=f"ghi{hi_i}"
                    )
                    col0 = int(st.hi_col_off[g]) + hc0 * 8
                    nn = (hc1 - hc0) * P
                    nc.gpsimd.dma_gather(
                        out_ap=bhi[:].rearrange("p (c e) -> p c e", e=st.D),
                        in_ap=x_hi,
                        idxs_ap=ihi_s[:, col0 : col0 + nn // 16],
                        num_idxs=nn,
                        num_idxs_reg=nn,
                        elem_size=st.D,
                        single_packet=False,
                        queue_num=next_q(nn),
                    )
                    for cc in range(hc0, hc1):
                        hi_tiles[cc] = (bhi, cc - hc0)

            for b in bs:
                total = int(st.C[b, 0] + st.C[b, 1])
                ps1 = p1pool.tile([P, P], f32, tag="ps1")
                k = 0
                for r, tmap in ((0, lo_tiles), (1, hi_tiles)):
                    for j in range(int(st.C[b, r])):
                        cc = st.call_base[(g, r, b)] + j
                        col = st.chunk_col[(g, r, b)] + j
                        t, tc = tmap[cc]
                        nc.tensor.matmul(
                            ps1[:],
                            lhsT=t[:, tc * st.D : (tc + 1) * st.D],
                            rhs=oh_slice(col),
                            start=(k == 0),
                            stop=(k == total - 1),
                        )
                        k += 1

                sums = spool.tile([P, P], bf16, tag="sums")
                nc.vector.tensor_copy(out=sums[:], in_=ps1[:])
                ps2 = p2pool.tile([P, st.D], f32, tag="ps2")
                nc.tensor.matmul(
                    ps2[:], lhsT=sums[:], rhs=wtb_s[:], start=True, stop=False
                )
                nc.tensor.matmul(
                    ps2[:],
                    lhsT=cntb_s[:1, b * P : (b + 1) * P],
                    rhs=browb_s[:1, :],
                    start=False,
                    stop=True,
                )
                of = opool.tile([P, st.D], f32, tag="of")
                nc.scalar.activation(
                    of[:], ps2[:], Act.Relu, scale=rs_s[:, b : b + 1]
                )
                nc.sync.dma_start(
                    out=out_t.ap()[b * P : (b + 1) * P, :], in_=of[:, :]
                )

    nc.compile()
    return nc


def emulate(x, edge_index, W, b):
    """Pure-numpy emulation of the device program (for validation)."""
    import ml_dtypes

    bf = ml_dtypes.bfloat16
    x = np.asarray(x, np.float32)
    st, per_core = _prep_structure(x.shape, edge_index)
    wt = np.ascontiguousarray(np.asarray(W, np.float32).T).astype(bf).astype(np.float32)
    brow = np.asarray(b, np.float32).astype(bf).astype(np.float32)
    outs = []
    xr = [x[:SPLIT], x[SPLIT:]]
    iota = np.arange(P, dtype=np.float32)
    for c in range(N_CORES):
        a = per_core[c]
        out_c = np.zeros((st.NPC, st.D), np.float32)
        for g, bs in enumerate(st.groups):
            gath = []
            for r, (arr, offs) in enumerate(
                ((a["idx_lo"], st.lo_col_off), (a["idx_hi"], st.hi_col_off))
            ):
                n = (st.n_lo, st.n_hi)[r][g]
                wrapped = arr[:16, int(offs[g]) : int(offs[g]) + n // 16]
                unwrapped = wrapped.T.reshape(-1).astype(np.int64)
                gath.append(
                    xr[r][unwrapped].astype(bf).astype(np.float32)
                    if n
                    else np.zeros((0, st.D), np.float32)
                )
            for bi in bs:
                S = np.zeros((st.D, P), np.float32)
                for r in (0, 1):
                    for j in range(int(st.C[bi, r])):
                        cc = st.call_base[(g, r, bi)] + j
                        col = st.chunk_col[(g, r, bi)] + j
                        got = gath[r][cc * P : (cc + 1) * P]  # [128e, D]
                        oh = (iota[None, :] == a["dl"][:, col][:, None]).astype(
                            np.float32
                        )
                        S += got.T @ oh
                Sb = S.astype(bf).astype(np.float32)
                cnt = a["cntrow"][0, bi * P : (bi + 1) * P].astype(bf).astype(np.float32)
                z = Sb.T @ wt + cnt[:, None] * brow[None, :]
                rs = a["rscols"][:, bi]
                oo = np.maximum(z * rs[:, None], 0.0)
                pb = int(st.perm[c, bi])
                nr = st.blk_rows[pb]
                out_c[pb * P : pb * P + nr] = oo[:nr]
        outs.append(out_c)
    return np.concatenate(outs, axis=0)[: x.shape[0]]


_RUN_INFO = {}


def _install_ntff_hook():
    """Recreate the antenv.axon_hooks NTFF profile hook via ctypes on the
    injected axon PJRT .so (the agent image's antenv lacks axon_hooks)."""
    import contextlib
    import ctypes
    import types

    try:
        from antenv.axon_hooks import get_axon_ntff_profile_hook  # noqa: F401

        return True
    except ImportError:
        pass

    so_path = "/opt/axon/libaxon_pjrt.so"
    if not os.path.exists(so_path):
        return False
    lib = ctypes.CDLL(so_path)
    if not hasattr(lib, "axon_start_nrt_profile"):
        return False
    lib.axon_start_nrt_profile.argtypes = [
        ctypes.POINTER(ctypes.c_int64),
        ctypes.c_size_t,
    ]
    lib.axon_start_nrt_profile.restype = ctypes.c_int64
    lib.axon_stop_nrt_profile.argtypes = [ctypes.c_char_p]
    lib.axon_stop_nrt_profile.restype = ctypes.c_int64

    @contextlib.contextmanager
    def _hook(output_dir, device_ids):
        import jax

        jax.devices()
        if device_ids:
            ids = (ctypes.c_int64 * len(device_ids))(*device_ids)
            rc = lib.axon_start_nrt_profile(ids, len(device_ids))
        else:
            rc = lib.axon_start_nrt_profile(None, 0)
        if rc != 0:
            raise RuntimeError(f"axon_start_nrt_profile rc={rc}")
        try:
            yield
        finally:
            n = lib.axon_stop_nrt_profile(str(output_dir).encode())
            print(f"ntff profile: {n} file(s) written to {output_dir}")

    mod = types.ModuleType("antenv.axon_hooks")
    mod.get_axon_ntff_profile_hook = lambda: _hook
    mod.set_axon_ntff_profile_hook = lambda h: None
    import antenv

    sys.modules["antenv.axon_hooks"] = mod
    antenv.axon_hooks = mod

    # avoid remote artifact uploads during profile post-processing
    from concourse import bass_utils

    bass_utils.upload_artifacts = lambda tmpdir: tmpdir
    return True


def kernel(x, edge_index, W, b, _trace=False):
    from concourse.bass_utils import run_bass_kernel_spmd

    import ml_dtypes as _mld

    x = np.ascontiguousarray(np.asarray(x, dtype=np.float32).astype(_mld.bfloat16))
    edge_index = np.asarray(edge_index)
    st, per_core = _prep_structure(x.shape, edge_index)
    wt = np.ascontiguousarray(np.asarray(W, np.float32).T.astype(_mld.bfloat16))
    brow = np.ascontiguousarray(
        np.asarray(b, np.float32).reshape(1, -1).astype(_mld.bfloat16)
    )
    import ml_dtypes

    iotaC = np.ascontiguousarray(
        np.tile(np.arange(P, dtype=np.float32), (P, OH_BATCH)).astype(
            ml_dtypes.bfloat16
        )
    )

    nc = _build_program(st)
    in_maps = []
    for c in range(N_CORES):
        a = per_core[c]
        in_maps.append(
            dict(
                x=x,
                idx_lo=a["idx_lo"],
                idx_hi=a["idx_hi"],
                dl=a["dl"],
                cntrow=a["cntrow"].astype(_mld.bfloat16),
                rscols=a["rscols"],
                iotaC=iotaC,
                brow=brow,
                wt=wt,
            )
        )
    if _trace:
        _trace = _install_ntff_hook()
    import tempfile

    tmpdir = tempfile.mkdtemp(prefix="gcn_bass_")
    try:
        res = run_bass_kernel_spmd(
            nc, in_maps, core_ids=list(range(N_CORES)), trace=_trace, tmpdir=tmpdir
        )
    except Exception:
        if not _trace:
            raise
        sys.stderr.write("trace run failed; retrying without trace\n")
        res = run_bass_kernel_spmd(nc, in_maps, core_ids=list(range(N_CORES)))
    _RUN_INFO["exec_time_ns"] = res.exec_time_ns
    _RUN_INFO["profile_json"] = res.profile_json
    _RUN_INFO["tmpdir"] = tmpdir
    out = np.zeros((st.N, st.D), np.float32)
    for c in range(N_CORES):
        oc = res.results[c]["out"]
        for j in range(st.NB):
            pb = int(st.perm[c, j])
            nr = st.blk_rows[pb]
            out[c * st.NPC + pb * P : c * st.NPC + pb * P + nr] = oc[
                j * P : j * P + nr
            ]
    return out
